# revision 59
# baseline (speedup 1.0000x reference)
# Trainium2 Bass kernel for nn_MinLoss_15229954032079.
#
# Math: loss = sum_b sum_s dist(p[b,s], g[b,match(b,s)]) / B, where
# dist is the euclidean distance between flattened [T*D] source signals
# and match is a greedy bipartite assignment on the [S,S] distance matrix.
#
# All pairwise distances derive from the 8x8 Gram matrix of the 8 flattened
# source vectors (4 prediction sources + 4 ground-truth sources) per batch:
#   d2[s,t] = G[s,s] + G[4+t,4+t] - 2*G[s,4+t]
#
# Strategy (one NeuronCore per batch element, 8 cores; the 33.7 MB HBM
# read streams at ~430 GB/s per core, which IS the per-core HBM share —
# so runtime = head + stream + exposed tail, and the optimization
# targets are the head/tail, not the stream; measured ~96us vs the
# 101.7us starting point):
#   - Stream p[b], g[b] into SBUF in TAPERED windows (512x7 + 128x4
#     timesteps): 512-step windows for the bulk, tiny windows at the end
#     so the exposed post-stream compute tail is a 128-step window's.
#     One fully contiguous DMA per (window, tensor) on the SWDGE cast
#     path (f32 HBM -> bf16 SBUF during the DMA). Window 0's p half goes
#     through Sync HWDGE (f32 landing) so the bandwidth-locked stream
#     starts a bit earlier. (1024-step windows were tried and REGRESSED:
#     same bandwidth, but their 10us copy / 11.5us matmul blocks smear
#     the slot-semaphore chains into a ~20us serial tail.)
#   - The shuffle into the blocked bf16 layout is split DVE/Activation:
#     within a window p lands before g, so the g-side copies (the
#     exposed segment that gates the matmuls) go mostly to DVE (fast);
#     p-side copies mostly to Activation (hidden under g's DMA); tail
#     (d=256) copies deferred on DVE.
#   - For each 128-column block, PSUM += block^T @ block on the PE (512
#     bf16 matmuls). PSUM entry (16j+u, 16j'+u) holds partial dots of
#     sources j,j'; summing the 16 u-diagonals on the host yields the
#     exact 8x8 Gram. Accumulation alternates two PSUM regions (psaA
#     even / psaB odd); psaA stops at window PSAA_LAST_W so its PSUM->
#     SBUF copy overlaps the stream, leaving only psaB's copy after the
#     final matmul. d=256 leftovers accumulate into per-TI-class PSUMs.
#   - All results land in one raw (non-tile) SBUF buffer; the single
#     output DMA is issued by a patched TileContext exit AFTER the
#     sem-waited drain, followed by a plain queue drain - no completion-
#     semaphore round trip on the critical path, no bass exit barrier
#     (the NRT end-of-program semaphore-clear pass has its own), and no
#     Bass-init barrier (no const APs are read). Each run executes a
#     freshly loaded NEFF, which is what makes the skipped clears safe.
#   - Tiny [4,4] greedy matching + final scalar reduction on host.

import numpy as np

B, T, S, D = 8, 4096, 4, 257
NCORES = 8
NJ = 8            # 4 pred sources + 4 gt sources

# Tapered windows: (timesteps, TI = timesteps/128). Sum of timesteps = T.
WINDOWS = [(512, 4)] * 7 + [(128, 1)] * 4
# psaA's accumulation group ends with this window (its writeback copy
# then overlaps the remaining stream); psaB takes everything after.
PSAA_LAST_W = 7
assert sum(tw for tw, _ in WINDOWS) == T
TI_CLASSES = sorted({ti for _, ti in WINDOWS}, reverse=True)  # [4, 1]

# Output layout: psa [128,128] in cols 0:128, then per-TI-class tail
# psums [8K, 8K] packed at col offsets in descending K order.
OUT_COLS = 128 + sum(8 * k for k in TI_CLASSES)

_cached_nc = None


def _make_light_exit(nc, gram_dram, outt):
    # Replaces TileContext._drain_and_barrier. Three tricks vs stock:
    #  - skip the per-semaphore clear pass and second barrier: every
    #    kernel() invocation executes a freshly loaded NEFF;
    #  - skip the bass-level all-engine barrier entirely: the NRT
    #    end-of-program epilogue (semaphore-file clear split across
    #    engines) begins with its own all-engine sync;
    #  - issue the output DMA AFTER the sem-waited drain, as a raw Sync
    #    instruction reading a non-tile SBUF buffer, followed by a plain
    #    queue drain. The drain's queue-empty test covers transfer
    #    completion without the DMA completion-semaphore round trip
    #    (~1us) that a tile-tracked output DMA pays.
    def _exit(self, tick_clock, wait_clock):
        from concourse.vector_clock import ScopedClock

        drain_inst = self.nc.sync.drain()
        wait_clock.add_sem_waits(
            drain_inst.ins, ScopedClock({None: tick_clock.global_clock})
        )
        out_sem = nc.alloc_semaphore("gram_out_sem")
        nc.sync.dma_start(out=gram_dram.ap(), in_=outt.ap()).then_inc(out_sem, 16)
        # No trailing queue drain: the NRT end-of-program epilogue injects
        # its own per-engine drain before the final NOTIFY, so the output
        # transfer completes during the ~6us semaphore-clear run instead
        # of gating the program end.
        popped = self.nc._tile_sem_poison_stack.pop()
        assert popped is self._sem_poison

    return _exit

def _build_nc():
    import concourse.bacc as bacc
    import concourse.bass as bass_mod
    import concourse.tile as tile
    from concourse import mybir

    # Bass.__init__ ends with four const-AP memsets on gpsimd and an
    # all-engine barrier fencing them. This kernel never reads a const
    # AP, and the tile clock protocol starts from zeroed semaphores, so
    # skip both: gpsimd's first DMA descriptor-gen (which gates the
    # bandwidth-locked HBM stream) starts ~1.5us earlier.
    _orig_aeb = bass_mod.Bass.all_engine_barrier
    _orig_ms = bass_mod.BassEitherVectorEngine.memset
    bass_mod.Bass.all_engine_barrier = lambda self, **kw: None
    bass_mod.BassEitherVectorEngine.memset = lambda self, ap, constant: None
    try:
        nc = bacc.Bacc(
            "TRN2", target_bir_lowering=False, debug=False, num_swdge_queues=1
        )
    finally:
        bass_mod.Bass.all_engine_barrier = _orig_aeb
        bass_mod.BassEitherVectorEngine.memset = _orig_ms
    p_dram = nc.dram_tensor("p", [T, S, D], mybir.dt.float32, kind="ExternalInput")
    g_dram = nc.dram_tensor("g", [T, S, D], mybir.dt.float32, kind="ExternalInput")
    # Single output: [psaA | psaB | psb tails] side by side; host sums
    # A+B and extracts the tails.
    gram_dram = nc.dram_tensor(
        "gram", [128, 256 + OUT_COLS - 128], mybir.dt.float32, kind="ExternalOutput"
    )
    outt = nc.alloc_sbuf_tensor("outt", [128, 256 + OUT_COLS - 128], mybir.dt.float32)

    orig_drain = tile.TileContext._drain_and_barrier
    tile.TileContext._drain_and_barrier = _make_light_exit(nc, gram_dram, outt)

    n_body_mm = sum(ti * 16 for _, ti in WINDOWS)
    first_of_class = {}
    last_of_class = {}
    for wi, (_, ti) in enumerate(WINDOWS):
        first_of_class.setdefault(ti, wi)
        last_of_class[ti] = wi

    with tile.TileContext(nc) as tc:
        with (
            tc.tile_pool(name="slab", bufs=6) as fpool,
            tc.tile_pool(name="w0p", bufs=1) as wpool,
            tc.tile_pool(name="blk16", bufs=3) as bpool,
            tc.tile_pool(name="psum", bufs=1, space="PSUM") as ppool,
        ):
            # psa split across two PSUM regions (even/odd matmuls) so that
            # psaA's accumulation can STOP early (window PSAA_LAST_W) and
            # its PSUM->SBUF writeback overlap the stream. Host sums A+B.
            psaA = ppool.tile([128, 128], mybir.dt.float32, tag="psaA")
            psaB = ppool.tile([128, 128], mybir.dt.float32, tag="psaB")
            psb = {}
            for k in TI_CLASSES:
                psb[k] = ppool.tile(
                    [NJ * k, NJ * k], mybir.dt.float32, name=f"psb{k}", tag=f"psb{k}"
                )

            mm_i = 0
            t0 = 0
            for wi, (tw, ti) in enumerate(WINDOWS):
                half = ti * S * D        # cols per tensor in raw HBM order
                cs = tw * D // 128       # 257*ti cols per source
                nblk = ti * 16           # full 128-col matmul blocks
                # [TW,S,D] slice -> [partition, ti, s, d]: partition p
                # covers times t0 + p*ti + ti_idx. One DMA per (window,
                # tensor): fully contiguous per partition (ti*4*257 f32).
                p_view = (
                    p_dram.ap()[t0 : t0 + tw].rearrange(
                        "(p ti) s d -> p ti s d", p=128, ti=ti
                    )
                )
                g_view = (
                    g_dram.ap()[t0 : t0 + tw].rearrange(
                        "(p ti) s d -> p ti s d", p=128, ti=ti
                    )
                )
                t0 += tw

                # slab holds the window in raw HBM order: [p-tensor | g-tensor],
                # per-partition column (ti, s, d) -> ti*1028 + s*257 + d.
                # The DMA is a plain contiguous copy that also casts
                # f32 -> bf16 (SWDGE path): the stream is read+write
                # combined bandwidth limited, so halving the write side
                # keeps the read at the HBM share.
                #
                # Window 0's p half goes through the Sync HWDGE instead
                # (f32 landing, no cast): the Sync engine is ready to
                # generate descriptors ~1.1us before gpsimd's SWDGE path,
                # so the whole HBM stream - whose duration is bandwidth-
                # locked - starts (and therefore ends) that much earlier.
                fslp = None
                if wi == 0:
                    fslp = wpool.tile([128, half], mybir.dt.float32)
                    nc.sync.dma_start(out=fslp[:], in_=p_view)
                    fsl = fpool.tile(
                        [128, half], mybir.dt.bfloat16, name=f"fsl{wi}", tag="fsl"
                    )
                    nc.gpsimd.dma_start(out=fsl[:, 0:half], in_=g_view)
                else:
                    fsl = fpool.tile(
                        [128, 2 * half], mybir.dt.bfloat16, name=f"fsl{wi}", tag="fsl"
                    )
                    nc.gpsimd.dma_start(out=fsl[:, 0:half], in_=p_view)
                    nc.gpsimd.dma_start(out=fsl[:, half : 2 * half], in_=g_view)

                wcols = 128 * nblk + NJ * ti
                wb = bpool.tile(
                    [128, wcols], mybir.dt.bfloat16, name=f"wb{wi}", tag="wb"
                )
                # per-source element order: q = (ti, dg, dl) — each block
                # is one ti and 16 consecutive d's per j, so copies move
                # 16-element contiguous runs on both sides. The leftover
                # d=256 gives ti tail cols per j.
                # body blocked col: (ti*16+dg)*128 + j*16 + dl
                wv = wb[:, 0 : 128 * nblk].rearrange(
                    "p (ti dg j dl) -> p j ti dg dl", ti=ti, dg=16, j=NJ, dl=16
                )
                # Copy-engine assignment. Within a window the p tensor
                # lands first, then g: the g-side copies are the exposed
                # serial segment that gates the matmuls. So: g bodies
                # (j=4,5,6) on DVE (fast), j=7 on Activation; p bodies
                # mostly on Activation (they hide under g's DMA); all tail
                # (d=256) copies deferred to DVE after the g bodies (they
                # only gate the tiny psb matmul).
                def srcj_of(j):
                    if wi == 0:
                        src = fslp if j < 4 else fsl[:, 0:half]
                    else:
                        off = 0 if j < 4 else half
                        src = fsl[:, off : off + half]
                    return src.rearrange("p (ti c) -> p ti c", ti=ti)[
                        :, :, (j % 4) * D : (j % 4 + 1) * D
                    ]

                def body_of(j):
                    return srcj_of(j)[:, :, 0:256].rearrange(
                        "p ti (dg dl) -> p ti dg dl", dl=16
                    )

                def tail_dst(j):
                    return wb[:, 128 * nblk + ti * j : 128 * nblk + ti * (j + 1)]

                # w0's p half is f32 (DVE copies run ~2x slower on it), so
                # give DVE two of its four p bodies; the p copies hide
                # under the g half's DMA anyway.
                #
                # Last window: the g-side copies after the final DMA are
                # the exposed critical path, and Activation's ~490ns fixed
                # cost per copy is 3x DVE's - so DVE takes ALL g bodies
                # and g tails; Activation keeps two p bodies and the
                # p-side tails (their data lands early).
                if wi == len(WINDOWS) - 1:
                    dve_bodies, dve_tails = (0, 3, 4, 5, 6, 7), (4, 5, 6, 7)
                elif wi == 0:
                    dve_bodies, dve_tails = (0, 1, 4, 5, 6), tuple(range(NJ))
                else:
                    dve_bodies, dve_tails = (0, 4, 5, 6), tuple(range(NJ))
                for j in range(NJ):
                    if j in dve_bodies:
                        nc.vector.tensor_copy(wv[:, j], body_of(j))
                    else:
                        nc.scalar.copy(wv[:, j], body_of(j))
                for j in range(NJ):
                    if j in dve_tails:
                        nc.vector.tensor_copy(tail_dst(j), srcj_of(j)[:, :, 256])
                    else:
                        nc.scalar.copy(tail_dst(j), srcj_of(j)[:, :, 256])

                def tail_mm():
                    tblk = wb[:, 128 * nblk : 128 * nblk + NJ * ti]
                    nc.tensor.matmul(
                        psb[ti][:],
                        tblk,
                        tblk,
                        start=(wi == first_of_class[ti]),
                        stop=(wi == last_of_class[ti]),
                    )

                # psaA stops accumulating at PSAA_LAST_W so its writeback
                # copy runs mid-stream; only psaB's copy remains after the
                # final matmul. Same for psb4 (its class ends at window 6).
                for r in range(nblk):
                    blk = wb[:, 128 * r : 128 * (r + 1)]
                    use_a = mm_i % 2 == 0 and wi <= PSAA_LAST_W
                    nc.tensor.matmul(
                        psaA[:] if use_a else psaB[:],
                        blk,
                        blk,
                        start=(mm_i < 2),
                        stop=(
                            (use_a and wi == PSAA_LAST_W and r >= nblk - 2)
                            or (not use_a and mm_i == n_body_mm - 1)
                        ),
                    )
                    mm_i += 1
                tail_mm()
                if wi == PSAA_LAST_W:
                    nc.vector.tensor_copy(outt.ap()[:, 0:128], psaA[:])
                if wi == last_of_class.get(4):
                    nc.scalar.copy(outt.ap()[0:32, 256:288], psb[4][:])

            # End-of-stream writeback: only psaB and psb1 remain (psaA and
            # psb4 were copied mid-stream). PSUM -> raw (non-tile) SBUF
            # buffer; the output DMA itself is issued by the patched exit
            # AFTER the final drain, so no completion semaphore sits on
            # the critical path.
            nc.scalar.copy(outt.ap()[0:8, 288:296], psb[1][:])
            nc.vector.tensor_copy(outt.ap()[:, 128:256], psaB[:])
    tile.TileContext._drain_and_barrier = orig_drain
    nc.compile()
    return nc


def _greedy_match_np(d):
    # replicate reference._greedy_match: repeated global argmin with
    # row/col masking; np.argmin matches jnp.argmin tie-breaking (first).
    s = d.shape[0]
    dm = d.astype(np.float32).copy()
    matches = np.zeros(s, np.int32)
    for _ in range(s):
        m = int(np.argmin(dm.reshape(-1)))
        r, c = divmod(m, s)
        matches[r] = c
        dm[r, :] = np.inf
        dm[:, c] = np.inf
    return matches


def _loss_from_gram(gram_list):
    total = 0.0
    for gram in gram_list:
        # body: G8[j,j'] = sum_u psa[16j+u, 16j'+u]; psa = psaA + psaB
        psa = gram[:, 0:128].astype(np.float64) + gram[:, 128:256].astype(np.float64)
        g8 = np.einsum("juku->jk", psa.reshape(8, 16, 8, 16))
        # tails: per TI-class K, psbK[kj+u, kj'+u] summed over u
        col = 256
        for k in TI_CLASSES:
            w = NJ * k
            pb = gram[0:w, col : col + w]
            g8 += np.einsum("juku->jk", pb.reshape(8, k, 8, k).astype(np.float64))
            col += w
        pn = np.diag(g8)[:4]
        gn = np.diag(g8)[4:]
        cr = g8[:4, 4:]
        d2 = pn[:, None] + gn[None, :] - 2.0 * cr
        dists = np.sqrt(np.maximum(d2, 0.0)).astype(np.float32)
        matches = _greedy_match_np(dists)
        total += float(dists[np.arange(4), matches].astype(np.float64).sum())
    return np.float32(total / B)


def kernel(**inputs):
    global _cached_nc
    preds = np.ascontiguousarray(inputs["predictions"], dtype=np.float32)
    gts = np.ascontiguousarray(inputs["ground_truths"], dtype=np.float32)
    assert preds.shape == (B, T, S, D) and gts.shape == (B, T, S, D)

    if _cached_nc is None:
        _cached_nc = _build_nc()
    nc = _cached_nc

    from concourse.bass_utils import run_bass_kernel_spmd

    in_maps = [{"p": preds[b], "g": gts[b]} for b in range(B)]
    res = run_bass_kernel_spmd(nc, in_maps, list(range(NCORES)))
    gram_list = [res.results[b]["gram"] for b in range(B)]
    return _loss_from_gram(gram_list)


# revision 63
# speedup vs baseline: 1.0117x; 1.0117x over previous
# Trainium2 Bass kernel for nn_MinLoss_15229954032079.
#
# Math: loss = sum_b sum_s dist(p[b,s], g[b,match(b,s)]) / B, where
# dist is the euclidean distance between flattened [T*D] source signals
# and match is a greedy bipartite assignment on the [S,S] distance matrix.
#
# All pairwise distances derive from the 8x8 Gram matrix of the 8 flattened
# source vectors (4 prediction sources + 4 ground-truth sources) per batch:
#   d2[s,t] = G[s,s] + G[4+t,4+t] - 2*G[s,4+t]
#
# Strategy (one NeuronCore per batch element, 8 cores; the 33.7 MB HBM
# read streams at ~430 GB/s per core, which IS the per-core HBM share —
# so runtime = head + stream + exposed tail, and the optimization
# targets are the head/tail, not the stream; measured ~96us vs the
# 101.7us starting point):
#   - Stream p[b], g[b] into SBUF in TAPERED windows (512x7 + 128x4
#     timesteps): 512-step windows for the bulk, tiny windows at the end
#     so the exposed post-stream compute tail is a 128-step window's.
#     One fully contiguous DMA per (window, tensor) on the SWDGE cast
#     path (f32 HBM -> bf16 SBUF during the DMA). Window 0's p half goes
#     through Sync HWDGE (f32 landing) so the bandwidth-locked stream
#     starts a bit earlier. (1024-step windows were tried and REGRESSED:
#     same bandwidth, but their 10us copy / 11.5us matmul blocks smear
#     the slot-semaphore chains into a ~20us serial tail.)
#   - The shuffle into the blocked bf16 layout is split DVE/Activation:
#     within a window p lands before g, so the g-side copies (the
#     exposed segment that gates the matmuls) go mostly to DVE (fast);
#     p-side copies mostly to Activation (hidden under g's DMA); tail
#     (d=256) copies deferred on DVE.
#   - For each 128-column block, PSUM += block^T @ block on the PE (512
#     bf16 matmuls). PSUM entry (16j+u, 16j'+u) holds partial dots of
#     sources j,j'; summing the 16 u-diagonals on the host yields the
#     exact 8x8 Gram. Accumulation alternates two PSUM regions (psaA
#     even / psaB odd); psaA stops at window PSAA_LAST_W so its PSUM->
#     SBUF copy overlaps the stream, leaving only psaB's copy after the
#     final matmul. d=256 leftovers accumulate into per-TI-class PSUMs.
#   - All results land in one raw (non-tile) SBUF buffer; the single
#     output DMA is issued by a patched TileContext exit AFTER the
#     sem-waited drain, followed by a plain queue drain - no completion-
#     semaphore round trip on the critical path, no bass exit barrier
#     (the NRT end-of-program semaphore-clear pass has its own), and no
#     Bass-init barrier (no const APs are read). Each run executes a
#     freshly loaded NEFF, which is what makes the skipped clears safe.
#   - Tiny [4,4] greedy matching + final scalar reduction on host.

import numpy as np

B, T, S, D = 8, 4096, 4, 257
NCORES = 8
NJ = 8            # 4 pred sources + 4 gt sources

# Tapered windows: (timesteps, TI = timesteps/128). Sum of timesteps = T.
WINDOWS = [(512, 4)] * 7 + [(128, 1)] * 4
# psaA's accumulation group ends with this window (its writeback copy
# then overlaps the remaining stream); psaB takes everything after.
PSAA_LAST_W = 7
# These windows' p halves go through Sync HWDGE (f32 landing, no cast)
# concurrently with the SWDGE cast stream: a second descriptor queue
# keeps the 16 shared DMA engines fed through per-instruction handoff
# gaps; f32 no-cast packets also run ~20% faster per engine.
F32P_WINDOWS = (3, 6)
assert sum(tw for tw, _ in WINDOWS) == T
TI_CLASSES = sorted({ti for _, ti in WINDOWS}, reverse=True)  # [4, 1]

# Output layout: psa [128,128] in cols 0:128, then per-TI-class tail
# psums [8K, 8K] packed at col offsets in descending K order.
OUT_COLS = 128 + sum(8 * k for k in TI_CLASSES)

_cached_nc = None


def _make_light_exit(nc, gram_dram, outt):
    # Replaces TileContext._drain_and_barrier. Three tricks vs stock:
    #  - skip the per-semaphore clear pass and second barrier: every
    #    kernel() invocation executes a freshly loaded NEFF;
    #  - skip the bass-level all-engine barrier entirely: the NRT
    #    end-of-program epilogue (semaphore-file clear split across
    #    engines) begins with its own all-engine sync;
    #  - issue the output DMA AFTER the sem-waited drain, as a raw Sync
    #    instruction reading a non-tile SBUF buffer, followed by a plain
    #    queue drain. The drain's queue-empty test covers transfer
    #    completion without the DMA completion-semaphore round trip
    #    (~1us) that a tile-tracked output DMA pays.
    def _exit(self, tick_clock, wait_clock):
        from concourse.vector_clock import ScopedClock

        drain_inst = self.nc.sync.drain()
        wait_clock.add_sem_waits(
            drain_inst.ins, ScopedClock({None: tick_clock.global_clock})
        )
        out_sem = nc.alloc_semaphore("gram_out_sem")
        nc.sync.dma_start(out=gram_dram.ap(), in_=outt.ap()).then_inc(out_sem, 16)
        # No trailing queue drain: the NRT end-of-program epilogue injects
        # its own per-engine drain before the final NOTIFY, so the output
        # transfer completes during the ~6us semaphore-clear run instead
        # of gating the program end.
        popped = self.nc._tile_sem_poison_stack.pop()
        assert popped is self._sem_poison

    return _exit

def _build_nc():
    import concourse.bacc as bacc
    import concourse.bass as bass_mod
    import concourse.tile as tile
    from concourse import mybir

    # Bass.__init__ ends with four const-AP memsets on gpsimd and an
    # all-engine barrier fencing them. This kernel never reads a const
    # AP, and the tile clock protocol starts from zeroed semaphores, so
    # skip both: gpsimd's first DMA descriptor-gen (which gates the
    # bandwidth-locked HBM stream) starts ~1.5us earlier.
    _orig_aeb = bass_mod.Bass.all_engine_barrier
    _orig_ms = bass_mod.BassEitherVectorEngine.memset
    bass_mod.Bass.all_engine_barrier = lambda self, **kw: None
    bass_mod.BassEitherVectorEngine.memset = lambda self, ap, constant: None
    try:
        nc = bacc.Bacc(
            "TRN2", target_bir_lowering=False, debug=False, num_swdge_queues=1
        )
    finally:
        bass_mod.Bass.all_engine_barrier = _orig_aeb
        bass_mod.BassEitherVectorEngine.memset = _orig_ms
    p_dram = nc.dram_tensor("p", [T, S, D], mybir.dt.float32, kind="ExternalInput")
    g_dram = nc.dram_tensor("g", [T, S, D], mybir.dt.float32, kind="ExternalInput")
    # Single output: [psaA | psaB | psb tails] side by side; host sums
    # A+B and extracts the tails.
    gram_dram = nc.dram_tensor(
        "gram", [128, 256 + OUT_COLS - 128], mybir.dt.float32, kind="ExternalOutput"
    )
    outt = nc.alloc_sbuf_tensor("outt", [128, 256 + OUT_COLS - 128], mybir.dt.float32)

    orig_drain = tile.TileContext._drain_and_barrier
    tile.TileContext._drain_and_barrier = _make_light_exit(nc, gram_dram, outt)

    n_body_mm = sum(ti * 16 for _, ti in WINDOWS)
    first_of_class = {}
    last_of_class = {}
    for wi, (_, ti) in enumerate(WINDOWS):
        first_of_class.setdefault(ti, wi)
        last_of_class[ti] = wi

    with tile.TileContext(nc) as tc:
        with (
            tc.tile_pool(name="slab", bufs=6) as fpool,
            tc.tile_pool(name="w0p", bufs=2) as wpool,
            tc.tile_pool(name="blk16", bufs=3) as bpool,
            tc.tile_pool(name="psum", bufs=1, space="PSUM") as ppool,
        ):
            # psa split across two PSUM regions (even/odd matmuls) so that
            # psaA's accumulation can STOP early (window PSAA_LAST_W) and
            # its PSUM->SBUF writeback overlap the stream. Host sums A+B.
            psaA = ppool.tile([128, 128], mybir.dt.float32, tag="psaA")
            psaB = ppool.tile([128, 128], mybir.dt.float32, tag="psaB")
            psb = {}
            for k in TI_CLASSES:
                psb[k] = ppool.tile(
                    [NJ * k, NJ * k], mybir.dt.float32, name=f"psb{k}", tag=f"psb{k}"
                )

            mm_i = 0
            t0 = 0
            for wi, (tw, ti) in enumerate(WINDOWS):
                half = ti * S * D        # cols per tensor in raw HBM order
                cs = tw * D // 128       # 257*ti cols per source
                nblk = ti * 16           # full 128-col matmul blocks
                # [TW,S,D] slice -> [partition, ti, s, d]: partition p
                # covers times t0 + p*ti + ti_idx. One DMA per (window,
                # tensor): fully contiguous per partition (ti*4*257 f32).
                p_view = (
                    p_dram.ap()[t0 : t0 + tw].rearrange(
                        "(p ti) s d -> p ti s d", p=128, ti=ti
                    )
                )
                g_view = (
                    g_dram.ap()[t0 : t0 + tw].rearrange(
                        "(p ti) s d -> p ti s d", p=128, ti=ti
                    )
                )
                t0 += tw

                # slab holds the window in raw HBM order: [p-tensor | g-tensor],
                # per-partition column (ti, s, d) -> ti*1028 + s*257 + d.
                # The DMA is a plain contiguous copy that also casts
                # f32 -> bf16 (SWDGE path): the stream is read+write
                # combined bandwidth limited, so halving the write side
                # keeps the read at the HBM share.
                #
                # Window 0's p half goes through the Sync HWDGE instead
                # (f32 landing, no cast): the Sync engine is ready to
                # generate descriptors ~1.1us before gpsimd's SWDGE path,
                # so the whole HBM stream - whose duration is bandwidth-
                # locked - starts (and therefore ends) that much earlier.
                fslp = None
                if wi == 0 or wi in F32P_WINDOWS:
                    fslp = wpool.tile(
                        [128, half], mybir.dt.float32, name=f"fslp{wi}", tag="pf32"
                    )
                    nc.sync.dma_start(out=fslp[:], in_=p_view)
                    fsl = fpool.tile(
                        [128, half], mybir.dt.bfloat16, name=f"fsl{wi}", tag="fsl"
                    )
                    nc.gpsimd.dma_start(out=fsl[:, 0:half], in_=g_view)
                else:
                    fsl = fpool.tile(
                        [128, 2 * half], mybir.dt.bfloat16, name=f"fsl{wi}", tag="fsl"
                    )
                    nc.gpsimd.dma_start(out=fsl[:, 0:half], in_=p_view)
                    nc.gpsimd.dma_start(out=fsl[:, half : 2 * half], in_=g_view)

                wcols = 128 * nblk + NJ * ti
                wb = bpool.tile(
                    [128, wcols], mybir.dt.bfloat16, name=f"wb{wi}", tag="wb"
                )
                # per-source element order: q = (ti, dg, dl) — each block
                # is one ti and 16 consecutive d's per j, so copies move
                # 16-element contiguous runs on both sides. The leftover
                # d=256 gives ti tail cols per j.
                # body blocked col: (ti*16+dg)*128 + j*16 + dl
                wv = wb[:, 0 : 128 * nblk].rearrange(
                    "p (ti dg j dl) -> p j ti dg dl", ti=ti, dg=16, j=NJ, dl=16
                )
                # Copy-engine assignment. Within a window the p tensor
                # lands first, then g: the g-side copies are the exposed
                # serial segment that gates the matmuls. So: g bodies
                # (j=4,5,6) on DVE (fast), j=7 on Activation; p bodies
                # mostly on Activation (they hide under g's DMA); all tail
                # (d=256) copies deferred to DVE after the g bodies (they
                # only gate the tiny psb matmul).
                def srcj_of(j):
                    if fslp is not None:
                        src = fslp if j < 4 else fsl[:, 0:half]
                    else:
                        off = 0 if j < 4 else half
                        src = fsl[:, off : off + half]
                    return src.rearrange("p (ti c) -> p ti c", ti=ti)[
                        :, :, (j % 4) * D : (j % 4 + 1) * D
                    ]

                def body_of(j):
                    return srcj_of(j)[:, :, 0:256].rearrange(
                        "p ti (dg dl) -> p ti dg dl", dl=16
                    )

                def tail_dst(j):
                    return wb[:, 128 * nblk + ti * j : 128 * nblk + ti * (j + 1)]

                # w0's p half is f32 (DVE copies run ~2x slower on it), so
                # give DVE two of its four p bodies; the p copies hide
                # under the g half's DMA anyway.
                #
                # Last window: the g-side copies after the final DMA are
                # the exposed critical path, and Activation's ~490ns fixed
                # cost per copy is 3x DVE's - so DVE takes ALL g bodies
                # and g tails; Activation keeps two p bodies and the
                # p-side tails (their data lands early).
                if wi == len(WINDOWS) - 1:
                    dve_bodies, dve_tails = (0, 3, 4, 5, 6, 7), (4, 5, 6, 7)
                elif wi == 0:
                    dve_bodies, dve_tails = (0, 1, 4, 5, 6), tuple(range(NJ))
                else:
                    dve_bodies, dve_tails = (0, 4, 5, 6), tuple(range(NJ))
                for j in range(NJ):
                    if j in dve_bodies:
                        nc.vector.tensor_copy(wv[:, j], body_of(j))
                    else:
                        nc.scalar.copy(wv[:, j], body_of(j))
                for j in range(NJ):
                    if j in dve_tails:
                        nc.vector.tensor_copy(tail_dst(j), srcj_of(j)[:, :, 256])
                    else:
                        nc.scalar.copy(tail_dst(j), srcj_of(j)[:, :, 256])

                def tail_mm():
                    tblk = wb[:, 128 * nblk : 128 * nblk + NJ * ti]
                    nc.tensor.matmul(
                        psb[ti][:],
                        tblk,
                        tblk,
                        start=(wi == first_of_class[ti]),
                        stop=(wi == last_of_class[ti]),
                    )

                # psaA stops accumulating at PSAA_LAST_W so its writeback
                # copy runs mid-stream; only psaB's copy remains after the
                # final matmul. Same for psb4 (its class ends at window 6).
                for r in range(nblk):
                    blk = wb[:, 128 * r : 128 * (r + 1)]
                    use_a = mm_i % 2 == 0 and wi <= PSAA_LAST_W
                    nc.tensor.matmul(
                        psaA[:] if use_a else psaB[:],
                        blk,
                        blk,
                        start=(mm_i < 2),
                        stop=(
                            (use_a and wi == PSAA_LAST_W and r >= nblk - 2)
                            or (not use_a and mm_i == n_body_mm - 1)
                        ),
                    )
                    mm_i += 1
                tail_mm()
                if wi == PSAA_LAST_W:
                    nc.vector.tensor_copy(outt.ap()[:, 0:128], psaA[:])
                if wi == last_of_class.get(4):
                    nc.scalar.copy(outt.ap()[0:32, 256:288], psb[4][:])

            # End-of-stream writeback: only psaB and psb1 remain (psaA and
            # psb4 were copied mid-stream). PSUM -> raw (non-tile) SBUF
            # buffer; the output DMA itself is issued by the patched exit
            # AFTER the final drain, so no completion semaphore sits on
            # the critical path.
            nc.scalar.copy(outt.ap()[0:8, 288:296], psb[1][:])
            nc.vector.tensor_copy(outt.ap()[:, 128:256], psaB[:])
    tile.TileContext._drain_and_barrier = orig_drain
    nc.compile()
    return nc


def _greedy_match_np(d):
    # replicate reference._greedy_match: repeated global argmin with
    # row/col masking; np.argmin matches jnp.argmin tie-breaking (first).
    s = d.shape[0]
    dm = d.astype(np.float32).copy()
    matches = np.zeros(s, np.int32)
    for _ in range(s):
        m = int(np.argmin(dm.reshape(-1)))
        r, c = divmod(m, s)
        matches[r] = c
        dm[r, :] = np.inf
        dm[:, c] = np.inf
    return matches


def _loss_from_gram(gram_list):
    total = 0.0
    for gram in gram_list:
        # body: G8[j,j'] = sum_u psa[16j+u, 16j'+u]; psa = psaA + psaB
        psa = gram[:, 0:128].astype(np.float64) + gram[:, 128:256].astype(np.float64)
        g8 = np.einsum("juku->jk", psa.reshape(8, 16, 8, 16))
        # tails: per TI-class K, psbK[kj+u, kj'+u] summed over u
        col = 256
        for k in TI_CLASSES:
            w = NJ * k
            pb = gram[0:w, col : col + w]
            g8 += np.einsum("juku->jk", pb.reshape(8, k, 8, k).astype(np.float64))
            col += w
        pn = np.diag(g8)[:4]
        gn = np.diag(g8)[4:]
        cr = g8[:4, 4:]
        d2 = pn[:, None] + gn[None, :] - 2.0 * cr
        dists = np.sqrt(np.maximum(d2, 0.0)).astype(np.float32)
        matches = _greedy_match_np(dists)
        total += float(dists[np.arange(4), matches].astype(np.float64).sum())
    return np.float32(total / B)


def kernel(**inputs):
    global _cached_nc
    preds = np.ascontiguousarray(inputs["predictions"], dtype=np.float32)
    gts = np.ascontiguousarray(inputs["ground_truths"], dtype=np.float32)
    assert preds.shape == (B, T, S, D) and gts.shape == (B, T, S, D)

    if _cached_nc is None:
        _cached_nc = _build_nc()
    nc = _cached_nc

    from concourse.bass_utils import run_bass_kernel_spmd

    in_maps = [{"p": preds[b], "g": gts[b]} for b in range(B)]
    res = run_bass_kernel_spmd(nc, in_maps, list(range(NCORES)))
    gram_list = [res.results[b]["gram"] for b in range(B)]
    return _loss_from_gram(gram_list)


# revision 64
# speedup vs baseline: 1.0121x; 1.0004x over previous
# Trainium2 Bass kernel for nn_MinLoss_15229954032079.
#
# Math: loss = sum_b sum_s dist(p[b,s], g[b,match(b,s)]) / B, where
# dist is the euclidean distance between flattened [T*D] source signals
# and match is a greedy bipartite assignment on the [S,S] distance matrix.
#
# All pairwise distances derive from the 8x8 Gram matrix of the 8 flattened
# source vectors (4 prediction sources + 4 ground-truth sources) per batch:
#   d2[s,t] = G[s,s] + G[4+t,4+t] - 2*G[s,4+t]
#
# Strategy (one NeuronCore per batch element, 8 cores; the 33.7 MB HBM
# read streams at ~430 GB/s per core, which IS the per-core HBM share —
# so runtime = head + stream + exposed tail, and the optimization
# targets are the head/tail, not the stream; measured ~96us vs the
# 101.7us starting point):
#   - Stream p[b], g[b] into SBUF in TAPERED windows (512x7 + 128x4
#     timesteps): 512-step windows for the bulk, tiny windows at the end
#     so the exposed post-stream compute tail is a 128-step window's.
#     One fully contiguous DMA per (window, tensor) on the SWDGE cast
#     path (f32 HBM -> bf16 SBUF during the DMA). Window 0's p half goes
#     through Sync HWDGE (f32 landing) so the bandwidth-locked stream
#     starts a bit earlier. (1024-step windows were tried and REGRESSED:
#     same bandwidth, but their 10us copy / 11.5us matmul blocks smear
#     the slot-semaphore chains into a ~20us serial tail.)
#   - The shuffle into the blocked bf16 layout is split DVE/Activation:
#     within a window p lands before g, so the g-side copies (the
#     exposed segment that gates the matmuls) go mostly to DVE (fast);
#     p-side copies mostly to Activation (hidden under g's DMA); tail
#     (d=256) copies deferred on DVE.
#   - For each 128-column block, PSUM += block^T @ block on the PE (512
#     bf16 matmuls). PSUM entry (16j+u, 16j'+u) holds partial dots of
#     sources j,j'; summing the 16 u-diagonals on the host yields the
#     exact 8x8 Gram. Accumulation alternates two PSUM regions (psaA
#     even / psaB odd); psaA stops at window PSAA_LAST_W so its PSUM->
#     SBUF copy overlaps the stream, leaving only psaB's copy after the
#     final matmul. d=256 leftovers accumulate into per-TI-class PSUMs.
#   - All results land in one raw (non-tile) SBUF buffer; the single
#     output DMA is issued by a patched TileContext exit AFTER the
#     sem-waited drain, followed by a plain queue drain - no completion-
#     semaphore round trip on the critical path, no bass exit barrier
#     (the NRT end-of-program semaphore-clear pass has its own), and no
#     Bass-init barrier (no const APs are read). Each run executes a
#     freshly loaded NEFF, which is what makes the skipped clears safe.
#   - Tiny [4,4] greedy matching + final scalar reduction on host.

import numpy as np

B, T, S, D = 8, 4096, 4, 257
NCORES = 8
NJ = 8            # 4 pred sources + 4 gt sources

# Tapered windows: (timesteps, TI = timesteps/128). Sum of timesteps = T.
WINDOWS = [(512, 4)] * 7 + [(128, 1)] * 4
# psaA's accumulation group ends with this window (its writeback copy
# then overlaps the remaining stream); psaB takes everything after.
PSAA_LAST_W = 7
assert sum(tw for tw, _ in WINDOWS) == T
TI_CLASSES = sorted({ti for _, ti in WINDOWS}, reverse=True)  # [4, 1]

# Output layout: psa [128,128] in cols 0:128, then per-TI-class tail
# psums [8K, 8K] packed at col offsets in descending K order.
OUT_COLS = 128 + sum(8 * k for k in TI_CLASSES)

_cached_nc = None


def _make_light_exit(nc, gram_dram, outt):
    # Replaces TileContext._drain_and_barrier. Three tricks vs stock:
    #  - skip the per-semaphore clear pass and second barrier: every
    #    kernel() invocation executes a freshly loaded NEFF;
    #  - skip the bass-level all-engine barrier entirely: the NRT
    #    end-of-program epilogue (semaphore-file clear split across
    #    engines) begins with its own all-engine sync;
    #  - issue the output DMA AFTER the sem-waited drain, as a raw Sync
    #    instruction reading a non-tile SBUF buffer, followed by a plain
    #    queue drain. The drain's queue-empty test covers transfer
    #    completion without the DMA completion-semaphore round trip
    #    (~1us) that a tile-tracked output DMA pays.
    def _exit(self, tick_clock, wait_clock):
        from concourse.vector_clock import ScopedClock

        drain_inst = self.nc.sync.drain()
        wait_clock.add_sem_waits(
            drain_inst.ins, ScopedClock({None: tick_clock.global_clock})
        )
        out_sem = nc.alloc_semaphore("gram_out_sem")
        nc.sync.dma_start(out=gram_dram.ap(), in_=outt.ap()).then_inc(out_sem, 16)
        # No trailing queue drain: the NRT end-of-program epilogue injects
        # its own per-engine drain before the final NOTIFY, so the output
        # transfer completes during the ~6us semaphore-clear run instead
        # of gating the program end.
        popped = self.nc._tile_sem_poison_stack.pop()
        assert popped is self._sem_poison

    return _exit

def _build_nc():
    import concourse.bacc as bacc
    import concourse.bass as bass_mod
    import concourse.tile as tile
    from concourse import mybir

    # Bass.__init__ ends with four const-AP memsets on gpsimd and an
    # all-engine barrier fencing them. This kernel never reads a const
    # AP, and the tile clock protocol starts from zeroed semaphores, so
    # skip both: gpsimd's first DMA descriptor-gen (which gates the
    # bandwidth-locked HBM stream) starts ~1.5us earlier.
    _orig_aeb = bass_mod.Bass.all_engine_barrier
    _orig_ms = bass_mod.BassEitherVectorEngine.memset
    bass_mod.Bass.all_engine_barrier = lambda self, **kw: None
    bass_mod.BassEitherVectorEngine.memset = lambda self, ap, constant: None
    try:
        nc = bacc.Bacc(
            "TRN2", target_bir_lowering=False, debug=False, num_swdge_queues=1
        )
    finally:
        bass_mod.Bass.all_engine_barrier = _orig_aeb
        bass_mod.BassEitherVectorEngine.memset = _orig_ms
    p_dram = nc.dram_tensor("p", [T, S, D], mybir.dt.float32, kind="ExternalInput")
    g_dram = nc.dram_tensor("g", [T, S, D], mybir.dt.float32, kind="ExternalInput")
    # Single output: [psaA | psaB | psb tails] side by side; host sums
    # A+B and extracts the tails.
    gram_dram = nc.dram_tensor(
        "gram", [128, 256 + OUT_COLS - 128], mybir.dt.float32, kind="ExternalOutput"
    )
    outt = nc.alloc_sbuf_tensor("outt", [128, 256 + OUT_COLS - 128], mybir.dt.float32)

    orig_drain = tile.TileContext._drain_and_barrier
    tile.TileContext._drain_and_barrier = _make_light_exit(nc, gram_dram, outt)

    n_body_mm = sum(ti * 16 for _, ti in WINDOWS)
    first_of_class = {}
    last_of_class = {}
    for wi, (_, ti) in enumerate(WINDOWS):
        first_of_class.setdefault(ti, wi)
        last_of_class[ti] = wi

    with tile.TileContext(nc) as tc:
        with (
            tc.tile_pool(name="slab", bufs=6) as fpool,
            tc.tile_pool(name="w0p", bufs=1) as wpool,
            tc.tile_pool(name="blk16", bufs=3) as bpool,
            tc.tile_pool(name="psum", bufs=1, space="PSUM") as ppool,
        ):
            # psa split across two PSUM regions (even/odd matmuls) so that
            # psaA's accumulation can STOP early (window PSAA_LAST_W) and
            # its PSUM->SBUF writeback overlap the stream. Host sums A+B.
            psaA = ppool.tile([128, 128], mybir.dt.float32, tag="psaA")
            psaB = ppool.tile([128, 128], mybir.dt.float32, tag="psaB")
            psb = {}
            for k in TI_CLASSES:
                psb[k] = ppool.tile(
                    [NJ * k, NJ * k], mybir.dt.float32, name=f"psb{k}", tag=f"psb{k}"
                )

            mm_i = 0
            t0 = 0
            for wi, (tw, ti) in enumerate(WINDOWS):
                half = ti * S * D        # cols per tensor in raw HBM order
                cs = tw * D // 128       # 257*ti cols per source
                nblk = ti * 16           # full 128-col matmul blocks
                # [TW,S,D] slice -> [partition, ti, s, d]: partition p
                # covers times t0 + p*ti + ti_idx. One DMA per (window,
                # tensor): fully contiguous per partition (ti*4*257 f32).
                p_view = (
                    p_dram.ap()[t0 : t0 + tw].rearrange(
                        "(p ti) s d -> p ti s d", p=128, ti=ti
                    )
                )
                g_view = (
                    g_dram.ap()[t0 : t0 + tw].rearrange(
                        "(p ti) s d -> p ti s d", p=128, ti=ti
                    )
                )
                t0 += tw

                # slab holds the window in raw HBM order: [p-tensor | g-tensor],
                # per-partition column (ti, s, d) -> ti*1028 + s*257 + d.
                # The DMA is a plain contiguous copy that also casts
                # f32 -> bf16 (SWDGE path): the stream is read+write
                # combined bandwidth limited, so halving the write side
                # keeps the read at the HBM share.
                #
                # Window 0's p half goes through the Sync HWDGE instead
                # (f32 landing, no cast): the Sync engine is ready to
                # generate descriptors ~1.1us before gpsimd's SWDGE path,
                # so the whole HBM stream - whose duration is bandwidth-
                # locked - starts (and therefore ends) that much earlier.
                fslp = None
                if wi == 0:
                    fslp = wpool.tile([128, half], mybir.dt.float32)
                    nc.sync.dma_start(out=fslp[:], in_=p_view)
                    fsl = fpool.tile(
                        [128, half], mybir.dt.bfloat16, name=f"fsl{wi}", tag="fsl"
                    )
                    nc.gpsimd.dma_start(out=fsl[:, 0:half], in_=g_view)
                else:
                    fsl = fpool.tile(
                        [128, 2 * half], mybir.dt.bfloat16, name=f"fsl{wi}", tag="fsl"
                    )
                    nc.gpsimd.dma_start(out=fsl[:, 0:half], in_=p_view)
                    nc.gpsimd.dma_start(out=fsl[:, half : 2 * half], in_=g_view)

                wcols = 128 * nblk + NJ * ti
                wb = bpool.tile(
                    [128, wcols], mybir.dt.bfloat16, name=f"wb{wi}", tag="wb"
                )
                # per-source element order: q = (ti, dg, dl) — each block
                # is one ti and 16 consecutive d's per j, so copies move
                # 16-element contiguous runs on both sides. The leftover
                # d=256 gives ti tail cols per j.
                # body blocked col: (ti*16+dg)*128 + j*16 + dl
                wv = wb[:, 0 : 128 * nblk].rearrange(
                    "p (ti dg j dl) -> p j ti dg dl", ti=ti, dg=16, j=NJ, dl=16
                )
                # Copy-engine assignment. Within a window the p tensor
                # lands first, then g: the g-side copies are the exposed
                # serial segment that gates the matmuls. So: g bodies
                # (j=4,5,6) on DVE (fast), j=7 on Activation; p bodies
                # mostly on Activation (they hide under g's DMA); all tail
                # (d=256) copies deferred to DVE after the g bodies (they
                # only gate the tiny psb matmul).
                def srcj_of(j):
                    if wi == 0:
                        src = fslp if j < 4 else fsl[:, 0:half]
                    else:
                        off = 0 if j < 4 else half
                        src = fsl[:, off : off + half]
                    return src.rearrange("p (ti c) -> p ti c", ti=ti)[
                        :, :, (j % 4) * D : (j % 4 + 1) * D
                    ]

                def body_of(j):
                    return srcj_of(j)[:, :, 0:256].rearrange(
                        "p ti (dg dl) -> p ti dg dl", dl=16
                    )

                def tail_dst(j):
                    return wb[:, 128 * nblk + ti * j : 128 * nblk + ti * (j + 1)]

                # w0's p half is f32 (DVE copies run ~2x slower on it), so
                # give DVE two of its four p bodies; the p copies hide
                # under the g half's DMA anyway.
                #
                # Last window: the g-side copies after the final DMA are
                # the exposed critical path, and Activation's ~490ns fixed
                # cost per copy is 3x DVE's - so DVE takes ALL g bodies
                # and g tails; Activation keeps two p bodies and the
                # p-side tails (their data lands early).
                if wi == len(WINDOWS) - 1:
                    dve_bodies, dve_tails = (0, 3, 4, 5, 6, 7), (4, 5, 6, 7)
                elif wi == 0:
                    dve_bodies, dve_tails = (0, 1, 4, 5, 6), tuple(range(NJ))
                else:
                    dve_bodies, dve_tails = (0, 4, 5, 6), tuple(range(NJ))
                for j in range(NJ):
                    if j in dve_bodies:
                        nc.vector.tensor_copy(wv[:, j], body_of(j))
                    else:
                        nc.scalar.copy(wv[:, j], body_of(j))
                for j in range(NJ):
                    if j in dve_tails:
                        nc.vector.tensor_copy(tail_dst(j), srcj_of(j)[:, :, 256])
                    else:
                        nc.scalar.copy(tail_dst(j), srcj_of(j)[:, :, 256])

                def tail_mm():
                    tblk = wb[:, 128 * nblk : 128 * nblk + NJ * ti]
                    nc.tensor.matmul(
                        psb[ti][:],
                        tblk,
                        tblk,
                        start=(wi == first_of_class[ti]),
                        stop=(wi == last_of_class[ti]),
                    )

                # psaA stops accumulating at PSAA_LAST_W so its writeback
                # copy runs mid-stream; only psaB's copy remains after the
                # final matmul. Same for psb4 (its class ends at window 6).
                for r in range(nblk):
                    blk = wb[:, 128 * r : 128 * (r + 1)]
                    use_a = mm_i % 2 == 0 and wi <= PSAA_LAST_W
                    nc.tensor.matmul(
                        psaA[:] if use_a else psaB[:],
                        blk,
                        blk,
                        start=(mm_i < 2),
                        stop=(
                            (use_a and wi == PSAA_LAST_W and r >= nblk - 2)
                            or (not use_a and mm_i == n_body_mm - 1)
                        ),
                    )
                    mm_i += 1
                tail_mm()
                if wi == PSAA_LAST_W:
                    nc.vector.tensor_copy(outt.ap()[:, 0:128], psaA[:])
                if wi == last_of_class.get(4):
                    nc.scalar.copy(outt.ap()[0:32, 256:288], psb[4][:])

            # End-of-stream writeback: only psaB and psb1 remain (psaA and
            # psb4 were copied mid-stream). PSUM -> raw (non-tile) SBUF
            # buffer; the output DMA itself is issued by the patched exit
            # AFTER the final drain, so no completion semaphore sits on
            # the critical path.
            nc.scalar.copy(outt.ap()[0:8, 288:296], psb[1][:])
            nc.vector.tensor_copy(outt.ap()[:, 128:256], psaB[:])
    tile.TileContext._drain_and_barrier = orig_drain
    nc.compile()
    return nc


def _greedy_match_np(d):
    # replicate reference._greedy_match: repeated global argmin with
    # row/col masking; np.argmin matches jnp.argmin tie-breaking (first).
    s = d.shape[0]
    dm = d.astype(np.float32).copy()
    matches = np.zeros(s, np.int32)
    for _ in range(s):
        m = int(np.argmin(dm.reshape(-1)))
        r, c = divmod(m, s)
        matches[r] = c
        dm[r, :] = np.inf
        dm[:, c] = np.inf
    return matches


def _loss_from_gram(gram_list):
    total = 0.0
    for gram in gram_list:
        # body: G8[j,j'] = sum_u psa[16j+u, 16j'+u]; psa = psaA + psaB
        psa = gram[:, 0:128].astype(np.float64) + gram[:, 128:256].astype(np.float64)
        g8 = np.einsum("juku->jk", psa.reshape(8, 16, 8, 16))
        # tails: per TI-class K, psbK[kj+u, kj'+u] summed over u
        col = 256
        for k in TI_CLASSES:
            w = NJ * k
            pb = gram[0:w, col : col + w]
            g8 += np.einsum("juku->jk", pb.reshape(8, k, 8, k).astype(np.float64))
            col += w
        pn = np.diag(g8)[:4]
        gn = np.diag(g8)[4:]
        cr = g8[:4, 4:]
        d2 = pn[:, None] + gn[None, :] - 2.0 * cr
        dists = np.sqrt(np.maximum(d2, 0.0)).astype(np.float32)
        matches = _greedy_match_np(dists)
        total += float(dists[np.arange(4), matches].astype(np.float64).sum())
    return np.float32(total / B)


def kernel(**inputs):
    global _cached_nc
    preds = np.ascontiguousarray(inputs["predictions"], dtype=np.float32)
    gts = np.ascontiguousarray(inputs["ground_truths"], dtype=np.float32)
    assert preds.shape == (B, T, S, D) and gts.shape == (B, T, S, D)

    if _cached_nc is None:
        _cached_nc = _build_nc()
    nc = _cached_nc

    from concourse.bass_utils import run_bass_kernel_spmd

    in_maps = [{"p": preds[b], "g": gts[b]} for b in range(B)]
    res = run_bass_kernel_spmd(nc, in_maps, list(range(NCORES)))
    gram_list = [res.results[b]["gram"] for b in range(B)]
    return _loss_from_gram(gram_list)
